# revision 15
# baseline (speedup 1.0000x reference)
"""Trainium2 Bass kernel for nn_Block_19095424598462 (dense transformer block
with talking-heads attention).  Data-parallel over batch: 8 cores x B=1.

Algebraic folding (host, fp64, exact):
  G_g = sum_h pre_w[h,g] wq_h wk_h^T / sqrt(KD)  (+ LN1 gamma/beta affine aug)
  V_h = sum_g post_w[h,g] wv_.. wo_..            (+ affine aug row)
  scores_g[t,s] = z_aug[t]^T G_g z_aug[s];  ctx = sum_s exp(scores) * [vt, 1]

v2 restructuring vs baseline:
  * Everything contracting over feature dims runs fp8 DoubleRow (2 rows per
    partition): scores, v-tilde, nh (query-side), fc1, fc2 AND ctx
    (probability x value) -- ctx via pair-packed-by-s e/vt tiles, produced
    for free by strided lhsT slices in the upstream matmuls.
  * Power-of-2 scales keep fp8 operands in range; descaling is folded into
    activation `scale` or the final scalar_tensor_tensor.
  * exp processes 2 PSUM banks per ACT instruction ([P, 2, 512]).
  * Softmax denominator via a 64.0-column in vt (numerator shares the scale).
  * LN z on gpsimd; single staged x DMA; chunk-level software pipeline:
    window c runs scores/exp(c) on PE/ACT while ctx(c-1) + LN2(c-1) + MLP
    batches fill PE/DVE; MLP gelus placed to need only 3 ACT table loads.
"""

import numpy as np
import ml_dtypes

import concourse.bass as bass
import concourse.mybir as mybir
import concourse.tile as tile
from concourse import bacc
from concourse.bass_utils import run_bass_kernel_spmd

F32 = mybir.dt.float32
BF16 = mybir.dt.bfloat16
FP8 = mybir.dt.float8e4
PM = mybir.MatmulPerfMode
AF = mybir.ActivationFunctionType
OP = mybir.AluOpType

# --- ACT table-set steering: Exp/Ln resolve uniquely to the shared set ------
_orig_get_tables = None


def _patched_tables(arch):
    tabs = _orig_get_tables(arch)
    keep = "natural_log_exp_and_others"
    if keep in tabs and AF.Exp in tabs[keep] and AF.Ln in tabs[keep]:
        for name, fns in tabs.items():
            if name != keep:
                fns.discard(AF.Exp)
                fns.discard(AF.Ln)
    return tabs


def _install_table_patch():
    global _orig_get_tables
    if _orig_get_tables is None:
        _orig_get_tables = bacc.get_activation_tables
        bacc.get_activation_tables = _patched_tables


P = 128
T = 2048
D = 192
DA = 193          # augmented (affine) contraction dim
KP = 97           # pair-packed contraction partitions (ceil(193/2))
NT = T // P       # 16 row tiles
TCH = 512         # t-chunk width
NCH = T // TCH    # 4 chunks
TSUB = TCH // P   # 4 subtiles per chunk
NW = T // 256     # 8 s-pair-blocks (256 s each)
HID = 768
NB = HID // 256   # 3 hid pair-blocks
NHEAD = 3
EPS = 1e-3

TRACE = False
LAST_RESULTS = None


def _pow2_scale(maxabs, target):
    """Largest power of 2 s such that maxabs * s <= target."""
    if maxabs <= 0:
        return 1.0
    return float(2.0 ** np.floor(np.log2(target / maxabs)))


def _prep_host(inp):
    f8 = np.float64
    wq, wk, wv, wo = (np.asarray(inp[k], f8) for k in ("wq", "wk", "wv", "wo"))
    pre_w, post_w = np.asarray(inp["pre_w"], f8), np.asarray(inp["post_w"], f8)
    g1, b1n = np.asarray(inp["gamma1"], f8), np.asarray(inp["beta1"], f8)
    g2, b2n = np.asarray(inp["gamma2"], f8), np.asarray(inp["beta2"], f8)
    w1, b1 = np.asarray(inp["w1"], f8), np.asarray(inp["b1"], f8)
    w2, b2 = np.asarray(inp["w2"], f8), np.asarray(inp["b2"], f8)
    KD = wq.shape[2]

    G = np.einsum("hg,dhk,ehk->gde", pre_w, wq, wk) / np.sqrt(KD)
    V = np.einsum("hg,dgk,gke->hde", post_w, wv, wo)
    b1p = b1 + b2n @ w1

    # augmented matrices [h, 193, ...]
    Ga = np.zeros((NHEAD, DA, DA), f8)
    for g in range(NHEAD):
        Gg = G[g]
        Ga[g, :D, :D] = (g1[:, None] * Gg) * g1[None, :]
        Ga[g, :D, D] = g1 * (Gg @ b1n)
        Ga[g, D, :D] = (b1n @ Gg) * g1
        Ga[g, D, D] = b1n @ Gg @ b1n
    Va = np.zeros((NHEAD, DA, D), f8)
    Va[:, :D, :] = g1[None, :, None] * V
    Va[:, D, :] = b1n @ V
    W1a = np.zeros((DA, HID), f8)
    W1a[:D] = g2[:, None] * w1
    W1a[D] = b1p

    # fp8 pow2 scales.  fp8e4m3(ieee) max finite = 240.
    # nh = Ga^T z_aug: bound nh rows by 6-sigma of row-norm (z~N(0,1)).
    g_rown = np.sqrt((Ga ** 2).sum(axis=1)).max()       # max over [g, m]
    gsc = _pow2_scale(6.0 * g_rown, 120.0)              # nh' range
    gsc = min(gsc, _pow2_scale(np.abs(Ga).max(), 120.0))
    v_coln = np.sqrt((Va ** 2).sum(axis=1)).max()
    vsc = _pow2_scale(6.0 * v_coln, 30.0)               # vt' range; also >=64 col fits
    vsc = min(vsc, _pow2_scale(np.abs(Va).max(), 120.0), 64.0)
    w1sc = _pow2_scale(np.abs(W1a).max(), 120.0)
    w2sc = _pow2_scale(np.abs(w2).max(), 120.0)

    e4 = ml_dtypes.float8_e4m3
    # gpk[p, i, g, mpos] = gsc * Ga[g, 2p+i, m]; column (m) axis permuted
    # even-first (mpos 0:97 <-> m even, 97:193 <-> m odd) and padded to 208
    # so DoubleRow LDWEIGHTS sees contiguous columns / 16-aligned pair stride.
    MP = 208
    gpk = np.zeros((P, 2, NHEAD, MP), f8)
    Gs = (Ga * gsc).transpose(1, 0, 2)                  # [193(d), g, m]
    for i in range(2):
        src = Gs[i:D:2]
        n = src.shape[0]
        gpk[:n, i, :, 0:KP] = src[:, :, 0::2]
        gpk[:n, i, :, KP:DA] = src[:, :, 1::2]
    gpk[KP - 1, 0, :, 0:KP] = Gs[D][:, 0::2]
    gpk[KP - 1, 0, :, KP:DA] = Gs[D][:, 1::2]
    # vpk[p, i, h, d] = vsc * Va[h, 2p+i, d]
    vpk = np.zeros((P, 2, NHEAD, D), f8)
    Vs = (Va * vsc).transpose(1, 0, 2)
    vpk[:KP - 1, 0] = Vs[0:D:2]
    vpk[:KP - 1, 1] = Vs[1:D:2]
    vpk[KP - 1, 0] = Vs[D]
    # w1pk[p, i, jpos] = w1sc * W1a[2p+i, j]; hid axis permuted per 256-block
    # (jpos 256B+128par+k <-> j = 256B+2k+par)
    w1pk = np.zeros((P, 2, HID), f8)
    W1s = W1a * w1sc
    w1j = np.zeros_like(W1s)
    for B in range(NB):
        w1j[:, 256 * B:256 * B + 128] = W1s[:, 256 * B:256 * (B + 1):2]
        w1j[:, 256 * B + 128:256 * (B + 1)] = W1s[:, 256 * B + 1:256 * (B + 1):2]
    w1pk[:KP - 1, 0] = w1j[0:D:2]
    w1pk[:KP - 1, 1] = w1j[1:D:2]
    w1pk[KP - 1, 0] = w1j[D]
    # w2pk[p, B, i, d] = w2sc * w2[256B+2p+i, d]
    w2pk = np.zeros((P, NB, 2, D), f8)
    w2s = w2 * w2sc
    for B in range(NB):
        blk = w2s[256 * B:256 * (B + 1)]
        w2pk[:, B, 0] = blk[0::2]
        w2pk[:, B, 1] = blk[1::2]

    weights = {
        "gpk": gpk.astype(e4),
        "vpk": vpk.astype(e4),
        "w1pk": w1pk.astype(e4),
        "w2pk": w2pk.astype(e4),
        "ident": np.eye(P, dtype=ml_dtypes.bfloat16),
    }
    has_b2 = bool(np.any(b2 != 0.0))
    if has_b2:
        weights["b2bc"] = np.broadcast_to(b2.astype(np.float32), (P, D)).copy()
    scales = (gsc, vsc, w1sc, w2sc)
    return weights, scales, has_b2


def _build(scales, has_b2):
    gsc, vsc, w1sc, w2sc = scales
    nc = bacc.Bacc("TRN2", target_bir_lowering=False, debug=False)

    MP = 208
    x_d = nc.declare_dram_parameter("x", [T, D], F32, isOutput=False)
    gpk_d = nc.declare_dram_parameter("gpk", [P, 2, NHEAD, MP], FP8, isOutput=False)
    vpk_d = nc.declare_dram_parameter("vpk", [P, 2, NHEAD, D], FP8, isOutput=False)
    w1_d = nc.declare_dram_parameter("w1pk", [P, 2, HID], FP8, isOutput=False)
    w2_d = nc.declare_dram_parameter("w2pk", [P, NB, 2, D], FP8, isOutput=False)
    id_d = nc.declare_dram_parameter("ident", [P, P], BF16, isOutput=False)
    if has_b2:
        b2_d = nc.declare_dram_parameter("b2bc", [P, D], F32, isOutput=False)
    y_d = nc.declare_dram_parameter("y", [T, D], F32, isOutput=True)

    from contextlib import ExitStack
    with tile.TileContext(nc) as tc, ExitStack() as ctx:
        singles = ctx.enter_context(tc.tile_pool(name="singles", bufs=1))
        work = ctx.enter_context(tc.tile_pool(name="work", bufs=4))
        e_pool = ctx.enter_context(tc.tile_pool(name="e_pool", bufs=2))
        ps2 = ctx.enter_context(tc.tile_pool(name="ps2", bufs=2, space="PSUM"))
        psc = ctx.enter_context(tc.tile_pool(name="psc", bufs=2, space="PSUM"))
        pst = ctx.enter_context(tc.tile_pool(name="pst", bufs=2, space="PSUM"))

        # ---- constants
        gsb = singles.tile([P, 2, NHEAD, MP], FP8)
        nc.sync.dma_start(out=gsb, in_=gpk_d.ap())
        vsb = singles.tile([P, 2, NHEAD, D], FP8)
        nc.sync.dma_start(out=vsb, in_=vpk_d.ap())
        w1sb = singles.tile([P, 2, HID], FP8)
        nc.sync.dma_start(out=w1sb, in_=w1_d.ap())
        w2sb = singles.tile([P, NB, 2, D], FP8)
        nc.sync.dma_start(out=w2sb, in_=w2_d.ap())
        ident = singles.tile([P, P], BF16)
        nc.sync.dma_start(out=ident, in_=id_d.ap())
        if has_b2:
            b2sb = singles.tile([P, D], F32)
            nc.sync.dma_start(out=b2sb, in_=b2_d.ap())
        eps_sb = singles.tile([P, 1], F32)
        nc.vector.memset(eps_sb, EPS)

        # ---- big SBUF state
        xall = singles.tile([P, NT, D], F32)
        zpk = singles.tile([P, 2, T], FP8)          # z_aug pair-packed by d
        nhpk = singles.tile([P, 2, NHEAD, T], FP8)  # nh' pair-packed by m
        vtpk = singles.tile([P, NHEAD, NW, 2, MP], FP8)
        n2t = [singles.tile([P, 2, TCH], FP8, tag=f"n2t{i}", name=f"n2t{i}")
               for i in range(2)]
        ht = [[singles.tile([P, 2, TCH], FP8, tag=f"ht{i}_{B}", name=f"ht{i}_{B}")
               for B in range(NB)] for i in range(2)]
        y1_tiles = [singles.tile([P, D], F32, tag=f"y1_{i}", name=f"y1_{i}")
                    for i in range(NT)]
        mv1 = singles.tile([P, NT, 2], F32)
        rstd1 = singles.tile([P, NT], F32)

        # affine rows / ones columns (set once)
        nc.vector.memset(zpk[KP - 1:KP, :, :], 0.0)
        nc.vector.memset(zpk[KP - 1:KP, 0, :], 1.0)
        nc.vector.memset(nhpk[KP - 1:KP, 1, :, :], 0.0)
        nc.vector.memset(vtpk[:, :, :, :, D:DA], vsc)
        for i in range(2):
            nc.vector.memset(n2t[i][KP - 1:KP, :, :], 0.0)
            nc.vector.memset(n2t[i][KP - 1:KP, 0, :], 1.0)

        # ---------------- helpers ----------------
        def ln_stats(src_ap, mv_slice):
            st = work.tile([P, 6], F32, tag="bnst")
            nc.vector.bn_stats(out=st, in_=src_ap)
            nc.vector.bn_aggr(out=mv_slice, in_=st)

        def ln_rstd(mv_ap, rstd_ap, n, tag):
            lnv = work.tile([P, n], F32, tag=tag)
            nc.scalar.activation(out=lnv, in_=mv_ap, func=AF.Ln, bias=eps_sb)
            nc.scalar.activation(out=rstd_ap, in_=lnv, func=AF.Exp, scale=-0.5)

        def z_and_pack(x_ap, mv_slice, rstd_ap, dst, col, ztag):
            """LN z (gpsimd, bf16) -> 2 strided transposes -> fp8 pair cast."""
            z = work.tile([P, D], BF16, tag=ztag)
            nc.gpsimd.tensor_scalar(
                out=z, in0=x_ap, scalar1=mv_slice[:, 0:1], scalar2=rstd_ap,
                op0=OP.subtract, op1=OP.mult)
            tp = pst.tile([P, 2, P], BF16, tag="tp")
            nc.tensor.transpose(tp[0:96, 0, :], z[:, 0:D:2], ident)
            nc.tensor.transpose(tp[0:96, 1, :], z[:, 1:D:2], ident)
            nc.vector.tensor_copy(out=dst[0:96, :, col:col + P], in_=tp[0:96, :, :])

        def emit_sc(c, g, w):
            """scores for (chunk c, head g, s-block w) -> psum pair + exp.

            The token axis is globally even/odd permuted within 256-blocks
            (tpos = 256w + 128*par + k <-> t = 256w + 2k + par), so the
            pair-packed layouts come from CONTIGUOUS lhsT column slices.
            """
            pp = ps2.tile([P, 2, TCH], F32, tag="p2")
            base = 256 * w
            csl = slice(c * TCH, (c + 1) * TCH)
            for par in range(2):
                nc.tensor.matmul(
                    pp[:, par, :],
                    lhsT=zpk[0:KP, :, base + 128 * par:base + 128 * par + P],
                    rhs=nhpk[0:KP, :, g, csl],
                    start=True, stop=True, perf_mode=PM.DoubleRow)
            et = e_pool.tile([P, 2, TCH], FP8, tag=f"e{g}_{w}")
            nc.scalar.activation(out=et, in_=pp, func=AF.Exp, scale=1.0 / gsc)
            return et

        def emit_vt(h, w):
            for par in range(2):
                pv = psc.tile([P, TCH], F32, tag="pc")
                base = 256 * w
                nc.tensor.matmul(
                    pv[:, 0:D],
                    lhsT=zpk[0:KP, :, base + 128 * par:base + 128 * par + P],
                    rhs=vsb[0:KP, :, h, :],
                    start=True, stop=True, perf_mode=PM.DoubleRow)
                nc.vector.tensor_copy(out=vtpk[:, h, w, par, 0:D], in_=pv[:, 0:D])

        def emit_nh(g, q, copy_eng):
            for par in range(2):
                mw = KP if par == 0 else KP - 1
                pn = psc.tile([P, TCH], F32, tag="pc")
                nc.tensor.matmul(
                    pn[0:mw, :],
                    lhsT=gsb[0:KP, :, g, KP * par:KP * par + mw],
                    rhs=zpk[0:KP, :, q * TCH:(q + 1) * TCH],
                    start=True, stop=True, perf_mode=PM.DoubleRow)
                dst = nhpk[0:mw, par, g, q * TCH:(q + 1) * TCH]
                if copy_eng == "act":
                    nc.scalar.copy(out=dst, in_=pn[0:mw, :])
                else:
                    nc.vector.tensor_copy(out=dst, in_=pn[0:mw, :])

        def emit_ctx_group(c, e_tiles, h, ts):
            """ctx accumulation over 8 s-blocks + y1 update for (h, ts)."""
            ti = c * TSUB + ts
            pc = psc.tile([P, TCH], F32, tag="pc")
            for w in range(NW):
                nc.tensor.matmul(
                    pc[:, 0:DA],
                    lhsT=e_tiles[(h, w)][:, :, ts * P:(ts + 1) * P],
                    rhs=vtpk[0:P, h, w, :, 0:DA],
                    start=(w == 0), stop=(w == NW - 1),
                    perf_mode=PM.DoubleRow)
            rc = work.tile([P, 1], F32, tag=f"rc{ts}")
            nc.vector.reciprocal(out=rc, in_=pc[:, D:DA])
            nc.vector.scalar_tensor_tensor(
                out=y1_tiles[ti], in0=pc[:, 0:D], scalar=rc,
                in1=(xall[:, ti, :] if h == 0 else y1_tiles[ti]),
                op0=OP.mult, op1=OP.add)

        def emit_ln2(c, mv2, rstd2):
            for ts in range(TSUB):
                ln_stats(y1_tiles[c * TSUB + ts], mv2[:, ts, :])
            ln_rstd(mv2[:, :, 1], rstd2, TSUB, f"ln2_{c % 2}")
            for ts in range(TSUB):
                z_and_pack(y1_tiles[c * TSUB + ts], mv2[:, ts, :],
                           rstd2[:, ts:ts + 1], n2t[c % 2], ts * P, f"z2_{ts}")

        def emit_fc1(c):
            for B in range(NB):
                pp = ps2.tile([P, 2, TCH], F32, tag="p2")
                for par in range(2):
                    nc.tensor.matmul(
                        pp[:, par, :],
                        lhsT=w1sb[0:KP, :, 256 * B + 128 * par:
                                  256 * B + 128 * par + P],
                        rhs=n2t[c % 2][0:KP, :, :],
                        start=True, stop=True, perf_mode=PM.DoubleRow)
                yield B, pp

        def emit_gelu(c, B, pp):
            nc.scalar.activation(out=ht[c % 2][B], in_=pp, func=AF.Gelu,
                                 scale=1.0 / w1sc)

        def emit_fc2(c):
            for ts in range(TSUB):
                ti = c * TSUB + ts
                pf = psc.tile([P, TCH], F32, tag="pc")
                for B in range(NB):
                    nc.tensor.matmul(
                        pf[:, 0:D],
                        lhsT=ht[c % 2][B][:, :, ts * P:(ts + 1) * P],
                        rhs=w2sb[:, B, :, :],
                        start=(B == 0), stop=(B == NB - 1),
                        perf_mode=PM.DoubleRow)
                ot = work.tile([P, D], F32, tag=f"ot{ts}")
                nc.vector.scalar_tensor_tensor(
                    out=ot, in0=pf[:, 0:D], scalar=1.0 / w2sc,
                    in1=y1_tiles[ti], op0=OP.mult, op1=OP.add)
                if has_b2:
                    nc.vector.tensor_tensor(out=ot, in0=ot, in1=b2sb, op=OP.add)
                nc.sync.dma_start(out=y_perm[ti // 2, ti % 2], in_=ot)

        # ---------------- Phase A: x load + LN1 + zpk ----------------
        # Global token permutation: tpos = 256w + 128*par + k <-> t = 256w
        # + 2k + par.  x is loaded permuted; y is stored back permuted; all
        # intermediate stages are token-oblivious.
        x_perm = x_d.ap().rearrange("(w k two) d -> w k two d", two=2, k=P)
        y_perm = y_d.ap().rearrange("(w k two) d -> w two k d", two=2, k=P)
        for w in range(NW):
            nc.sync.dma_start(out=xall[:, 2 * w:2 * w + 2, :], in_=x_perm[w])
        for g4 in range(4):
            for i in range(4 * g4, 4 * g4 + 4):
                ln_stats(xall[:, i, :], mv1[:, i, :])
            ln_rstd(mv1[:, 4 * g4:4 * (g4 + 1), 1],
                    rstd1[:, 4 * g4:4 * (g4 + 1)], 4, f"ln1_{g4 % 2}")
            for i in range(4 * g4, 4 * g4 + 4):
                z_and_pack(xall[:, i, :], mv1[:, i, :], rstd1[:, i:i + 1],
                           zpk, i * P, f"z1_{i % 4}")

        # ---------------- Phase B: nh for g0 (rest woven into W0) -----
        for q in range(NCH):
            emit_nh(0, q, "act" if q % 2 == 0 else "dve")

        # ---------------- chunk windows ----------------
        mv2 = [work.tile([P, TSUB, 2], F32, tag=f"mv2_{i}", name=f"mv2_{i}")
               for i in range(2)]
        rstd2 = [work.tile([P, TSUB], F32, tag=f"rs2_{i}", name=f"rs2_{i}")
                 for i in range(2)]
        e_tiles_by_chunk = {}

        # W0: scores/exp(0); weave nh(g1,g2) + vt jobs
        fill_w0 = ([("nh", 1, q) for q in range(NCH)]
                   + [("nh", 2, q) for q in range(NCH)]
                   + [("vt", h, w) for h in range(NHEAD) for w in range(NW)])
        et0 = {}
        fi = 0
        for g in range(NHEAD):
            for w in range(NW):
                et0[(g, w)] = emit_sc(0, g, w)
                take = 2 if fi < 8 else 1
                for _ in range(take):
                    if fi < len(fill_w0):
                        job = fill_w0[fi]; fi += 1
                        if job[0] == "nh":
                            emit_nh(job[1], job[2], "dve" if fi % 2 else "act")
                        else:
                            emit_vt(job[1], job[2])
        while fi < len(fill_w0):
            job = fill_w0[fi]; fi += 1
            if job[0] == "nh":
                emit_nh(job[1], job[2], "dve")
            else:
                emit_vt(job[1], job[2])
        e_tiles_by_chunk[0] = et0

        # W1..W3: scores/exp(c) woven with ctx(c-1); then LN2(c-1); MLP spread
        for c in range(1, NCH):
            etc = {}
            prev = e_tiles_by_chunk[c - 1]
            ctx_groups = [(h, ts) for h in range(NHEAD) for ts in range(TSUB)]
            ci = 0
            sc_cnt = 0
            for g in range(NHEAD):
                for w in range(NW):
                    etc[(g, w)] = emit_sc(c, g, w)
                    sc_cnt += 1
                    if sc_cnt % 2 == 0 and ci < len(ctx_groups):
                        h, ts = ctx_groups[ci]; ci += 1
                        emit_ctx_group(c - 1, prev, h, ts)
            while ci < len(ctx_groups):
                h, ts = ctx_groups[ci]; ci += 1
                emit_ctx_group(c - 1, prev, h, ts)
            e_tiles_by_chunk[c] = etc
            del e_tiles_by_chunk[c - 1]

            emit_ln2(c - 1, mv2[(c - 1) % 2], rstd2[(c - 1) % 2])

            if c == 2:
                # MLP(0): fc1+gelu now; fc2(0) next window
                for B, pp in emit_fc1(0):
                    emit_gelu(0, B, pp)
                for B, pp in emit_fc1(1):
                    emit_gelu(1, B, pp)
            if c == 3:
                emit_fc2(0)
                emit_fc2(1)

        # tail: ctx(3), LN2(3), MLP(2), MLP(3)
        prev = e_tiles_by_chunk[3]
        for h in range(NHEAD):
            for ts in range(TSUB):
                emit_ctx_group(3, prev, h, ts)
        emit_ln2(3, mv2[1], rstd2[1])
        for B, pp in emit_fc1(2):
            emit_gelu(2, B, pp)
        emit_fc2(2)
        for B, pp in emit_fc1(3):
            emit_gelu(3, B, pp)
        emit_fc2(3)

    nc.finalize()
    return nc


_module_cache = {}


def kernel(**inputs):
    global LAST_RESULTS
    x = np.ascontiguousarray(np.asarray(inputs["x"], np.float32))
    B = x.shape[0]
    assert x.shape == (B, T, D) and B == 8

    weights, scales, has_b2 = _prep_host(inputs)

    _install_table_patch()
    key = (scales, has_b2)
    if key not in _module_cache:
        _module_cache[key] = _build(scales, has_b2)
    nc = _module_cache[key]

    in_maps = [dict(weights, x=x[b]) for b in range(B)]
    res = run_bass_kernel_spmd(nc, in_maps, core_ids=list(range(B)), trace=TRACE)
    LAST_RESULTS = res
    out = np.stack([np.asarray(res.results[b]["y"], np.float32) for b in range(B)])
    return out


# revision 26
# speedup vs baseline: 1.5481x; 1.5481x over previous
"""Trainium2 Bass kernel for nn_Block_19095424598462 (dense transformer block
with talking-heads attention).  Data-parallel over batch: 8 cores x B=1.

Algebraic folding (host, fp64, exact):
  G_g = sum_h pre_w[h,g] wq_h wk_h^T / sqrt(KD)  (+ LN1 gamma/beta affine aug)
  V_h = sum_g post_w[h,g] wv_.. wo_..            (+ affine aug row)
  scores_g[t,s] = z_aug[t]^T G_g z_aug[s];  ctx = sum_s exp(scores) * [vt, 1]

v2 restructuring vs baseline:
  * Everything contracting over feature dims runs fp8 DoubleRow (2 rows per
    partition): scores, v-tilde, nh (query-side), fc1, fc2 AND ctx
    (probability x value) -- ctx via pair-packed-by-s e/vt tiles, produced
    for free by strided lhsT slices in the upstream matmuls.
  * Power-of-2 scales keep fp8 operands in range; descaling is folded into
    activation `scale` or the final scalar_tensor_tensor.
  * exp processes 2 PSUM banks per ACT instruction ([P, 2, 512]).
  * Softmax denominator via a 64.0-column in vt (numerator shares the scale).
  * LN z on gpsimd; single staged x DMA; chunk-level software pipeline:
    window c runs scores/exp(c) on PE/ACT while ctx(c-1) + LN2(c-1) + MLP
    batches fill PE/DVE; MLP gelus placed to need only 3 ACT table loads.
"""

import numpy as np
import ml_dtypes

import concourse.bass as bass
import concourse.mybir as mybir
import concourse.tile as tile
from concourse import bacc
from concourse.bass_utils import run_bass_kernel_spmd

F32 = mybir.dt.float32
BF16 = mybir.dt.bfloat16
FP8 = mybir.dt.float8e4
PM = mybir.MatmulPerfMode
AF = mybir.ActivationFunctionType
OP = mybir.AluOpType

# --- ACT table-set steering: Exp/Ln resolve uniquely to the shared set ------
_orig_get_tables = None


def _patched_tables(arch):
    tabs = _orig_get_tables(arch)
    keep = "natural_log_exp_and_others"
    if keep in tabs and AF.Exp in tabs[keep] and AF.Ln in tabs[keep]:
        for name, fns in tabs.items():
            if name != keep:
                fns.discard(AF.Exp)
                fns.discard(AF.Ln)
    return tabs


def _install_table_patch():
    global _orig_get_tables
    if _orig_get_tables is None:
        _orig_get_tables = bacc.get_activation_tables
        bacc.get_activation_tables = _patched_tables


P = 128
T = 2048
D = 192
DA = 193          # augmented (affine) contraction dim
KP = 97           # pair-packed contraction partitions (ceil(193/2))
NT = T // P       # 16 row tiles
TCH = 512         # t-chunk width
NCH = T // TCH    # 4 chunks
TSUB = TCH // P   # 4 subtiles per chunk
NW = T // 256     # 8 s-pair-blocks (256 s each)
HID = 768
NB = HID // 256   # 3 hid pair-blocks
NHEAD = 3
EPS = 1e-3

TRACE = False
LAST_RESULTS = None


def _pow2_scale(maxabs, target):
    """Largest power of 2 s such that maxabs * s <= target."""
    if maxabs <= 0:
        return 1.0
    return float(2.0 ** np.floor(np.log2(target / maxabs)))


def _prep_host(inp):
    f8 = np.float64
    wq, wk, wv, wo = (np.asarray(inp[k], f8) for k in ("wq", "wk", "wv", "wo"))
    pre_w, post_w = np.asarray(inp["pre_w"], f8), np.asarray(inp["post_w"], f8)
    g1, b1n = np.asarray(inp["gamma1"], f8), np.asarray(inp["beta1"], f8)
    g2, b2n = np.asarray(inp["gamma2"], f8), np.asarray(inp["beta2"], f8)
    w1, b1 = np.asarray(inp["w1"], f8), np.asarray(inp["b1"], f8)
    w2, b2 = np.asarray(inp["w2"], f8), np.asarray(inp["b2"], f8)
    KD = wq.shape[2]

    G = np.einsum("hg,dhk,ehk->gde", pre_w, wq, wk) / np.sqrt(KD)
    V = np.einsum("hg,dgk,gke->hde", post_w, wv, wo)
    b1p = b1 + b2n @ w1

    # augmented matrices [h, 193, ...]
    Ga = np.zeros((NHEAD, DA, DA), f8)
    for g in range(NHEAD):
        Gg = G[g]
        Ga[g, :D, :D] = (g1[:, None] * Gg) * g1[None, :]
        Ga[g, :D, D] = g1 * (Gg @ b1n)
        Ga[g, D, :D] = (b1n @ Gg) * g1
        Ga[g, D, D] = b1n @ Gg @ b1n
    Va = np.zeros((NHEAD, DA, D), f8)
    Va[:, :D, :] = g1[None, :, None] * V
    Va[:, D, :] = b1n @ V
    W1a = np.zeros((DA, HID), f8)
    W1a[:D] = g2[:, None] * w1
    W1a[D] = b1p

    # fp8 pow2 scales.  fp8e4m3(ieee) max finite = 240.
    # nh = Ga^T z_aug: bound nh rows by 6-sigma of row-norm (z~N(0,1)).
    g_rown = np.sqrt((Ga ** 2).sum(axis=1)).max()       # max over [g, m]
    gsc = _pow2_scale(6.0 * g_rown, 120.0)              # nh' range
    gsc = min(gsc, _pow2_scale(np.abs(Ga).max(), 120.0))
    v_coln = np.sqrt((Va ** 2).sum(axis=1)).max()
    vsc = _pow2_scale(6.0 * v_coln, 30.0)               # vt' range; also >=64 col fits
    vsc = min(vsc, _pow2_scale(np.abs(Va).max(), 120.0), 64.0)
    w1sc = _pow2_scale(np.abs(W1a).max(), 120.0)
    w2sc = _pow2_scale(np.abs(w2).max(), 120.0)

    e4 = ml_dtypes.float8_e4m3
    # gpk[p, i, g, mpos] = gsc * Ga[g, 2p+i, m]; column (m) axis permuted
    # even-first (mpos 0:97 <-> m even, 97:193 <-> m odd) and padded to 208
    # so DoubleRow LDWEIGHTS sees contiguous columns / 16-aligned pair stride.
    MP = 208
    gpk = np.zeros((P, 2, NHEAD, MP), f8)
    Gs = (Ga * gsc).transpose(1, 0, 2)                  # [193(d), g, m]
    for i in range(2):
        src = Gs[i:D:2]
        n = src.shape[0]
        gpk[:n, i, :, 0:KP] = src[:, :, 0::2]
        gpk[:n, i, :, KP:DA] = src[:, :, 1::2]
    gpk[KP - 1, 0, :, 0:KP] = Gs[D][:, 0::2]
    gpk[KP - 1, 0, :, KP:DA] = Gs[D][:, 1::2]
    # vpk[p, i, h, d] = vsc * Va[h, 2p+i, d]
    vpk = np.zeros((P, 2, NHEAD, D), f8)
    Vs = (Va * vsc).transpose(1, 0, 2)
    vpk[:KP - 1, 0] = Vs[0:D:2]
    vpk[:KP - 1, 1] = Vs[1:D:2]
    vpk[KP - 1, 0] = Vs[D]
    # w1pk[p, i, jpos] = w1sc * W1a[2p+i, j]; hid axis permuted per 256-block
    # (jpos 256B+128par+k <-> j = 256B+2k+par)
    w1pk = np.zeros((P, 2, HID), f8)
    W1s = W1a * w1sc
    w1j = np.zeros_like(W1s)
    for B in range(NB):
        w1j[:, 256 * B:256 * B + 128] = W1s[:, 256 * B:256 * (B + 1):2]
        w1j[:, 256 * B + 128:256 * (B + 1)] = W1s[:, 256 * B + 1:256 * (B + 1):2]
    w1pk[:KP - 1, 0] = w1j[0:D:2]
    w1pk[:KP - 1, 1] = w1j[1:D:2]
    w1pk[KP - 1, 0] = w1j[D]
    # w2pk[p, B, i, d] = w2sc * w2[256B+2p+i, d]
    w2pk = np.zeros((P, NB, 2, D), f8)
    w2s = w2 * w2sc
    for B in range(NB):
        blk = w2s[256 * B:256 * (B + 1)]
        w2pk[:, B, 0] = blk[0::2]
        w2pk[:, B, 1] = blk[1::2]

    # affine-row constants, DMA'd into partition-96 slices (cheaper than
    # single-partition memsets, which cost free_size cycles on DVE):
    # zpk[96,:,:] = [ones(T), zeros(T)]; nhpk[96,1,:,:] = zeros(3T);
    # n2t{0,1}[96,:,:] = [ones(512), zeros(512)]
    affc = np.zeros((1, 2 * T + NHEAD * T + 2 * 2 * 512), f8)
    affc[0, 0:T] = 1.0
    affc[0, 2 * T + NHEAD * T:2 * T + NHEAD * T + 512] = 1.0
    affc[0, 2 * T + NHEAD * T + 1024:2 * T + NHEAD * T + 1536] = 1.0

    weights = {
        "gpk": gpk.astype(e4),
        "vpk": vpk.astype(e4),
        "w1pk": w1pk.astype(e4),
        "w2pk": w2pk.astype(e4),
        "affc": affc.astype(e4),
        "ident": np.eye(P, dtype=ml_dtypes.bfloat16),
    }
    has_b2 = bool(np.any(b2 != 0.0))
    if has_b2:
        weights["b2bc"] = np.broadcast_to(b2.astype(np.float32), (P, D)).copy()
    scales = (gsc, vsc, w1sc, w2sc)
    return weights, scales, has_b2


def _build(scales, has_b2):
    gsc, vsc, w1sc, w2sc = scales
    nc = bacc.Bacc("TRN2", target_bir_lowering=False, debug=False)

    MP = 208
    x_d = nc.declare_dram_parameter("x", [T, D], F32, isOutput=False)
    gpk_d = nc.declare_dram_parameter("gpk", [P, 2, NHEAD, MP], FP8, isOutput=False)
    vpk_d = nc.declare_dram_parameter("vpk", [P, 2, NHEAD, D], FP8, isOutput=False)
    w1_d = nc.declare_dram_parameter("w1pk", [P, 2, HID], FP8, isOutput=False)
    w2_d = nc.declare_dram_parameter("w2pk", [P, NB, 2, D], FP8, isOutput=False)
    affc_d = nc.declare_dram_parameter(
        "affc", [1, 2 * T + NHEAD * T + 2 * 2 * 512], FP8, isOutput=False)
    id_d = nc.declare_dram_parameter("ident", [P, P], BF16, isOutput=False)
    if has_b2:
        b2_d = nc.declare_dram_parameter("b2bc", [P, D], F32, isOutput=False)
    y_d = nc.declare_dram_parameter("y", [T, D], F32, isOutput=True)

    from contextlib import ExitStack
    with tile.TileContext(nc) as tc, ExitStack() as ctx:
        singles = ctx.enter_context(tc.tile_pool(name="singles", bufs=1))
        work = ctx.enter_context(tc.tile_pool(name="work", bufs=4))
        e_pool = ctx.enter_context(tc.tile_pool(name="e_pool", bufs=2))
        ps2 = ctx.enter_context(tc.tile_pool(name="ps2", bufs=2, space="PSUM"))
        psc = ctx.enter_context(tc.tile_pool(name="psc", bufs=2, space="PSUM"))
        pst = ctx.enter_context(tc.tile_pool(name="pst", bufs=2, space="PSUM"))

        # ---- constants
        gsb = singles.tile([P, 2, NHEAD, MP], FP8)
        nc.sync.dma_start(out=gsb, in_=gpk_d.ap())
        vsb = singles.tile([P, 2, NHEAD, D], FP8)
        nc.sync.dma_start(out=vsb, in_=vpk_d.ap())
        w1sb = singles.tile([P, 2, HID], FP8)
        nc.sync.dma_start(out=w1sb, in_=w1_d.ap())
        w2sb = singles.tile([P, NB, 2, D], FP8)
        nc.sync.dma_start(out=w2sb, in_=w2_d.ap())
        ident = singles.tile([P, P], BF16)
        nc.sync.dma_start(out=ident, in_=id_d.ap())
        if has_b2:
            b2sb = singles.tile([P, D], F32)
            nc.sync.dma_start(out=b2sb, in_=b2_d.ap())

        # ---- big SBUF state
        xall = singles.tile([P, NT, D], F32)
        zpk = singles.tile([P, 2, T], FP8)          # z_aug pair-packed by d
        nhpk = singles.tile([P, 2, NHEAD, T], FP8)  # nh' pair-packed by m
        vtpk = singles.tile([P, NHEAD, NW, 2, MP], FP8)
        n2t = [singles.tile([P, 2, TCH], FP8, tag=f"n2t{i}", name=f"n2t{i}")
               for i in range(2)]
        ht = [[singles.tile([P, 2, TCH], FP8, tag=f"ht{i}_{B}", name=f"ht{i}_{B}")
               for B in range(NB)] for i in range(2)]
        y1_tiles = [singles.tile([P, D], F32, tag=f"y1_{i}", name=f"y1_{i}")
                    for i in range(NT)]
        mv1 = singles.tile([P, NT, 2], F32)
        rstd1 = singles.tile([P, NT], F32)

        # affine rows via DMA; ones-column (strided, small) via memset
        aff = affc_d.ap()
        nc.sync.dma_start(
            out=zpk[KP - 1:KP, :, :],
            in_=aff[0:1, 0:2 * T].rearrange("p (i t) -> p i t", i=2))
        nc.sync.dma_start(
            out=nhpk[KP - 1:KP, 1, :, :],
            in_=aff[0:1, 2 * T:2 * T + NHEAD * T].rearrange(
                "p (g t) -> p g t", g=NHEAD))
        for i in range(2):
            off = 2 * T + NHEAD * T + i * 1024
            nc.sync.dma_start(
                out=n2t[i][KP - 1:KP, :, :],
                in_=aff[0:1, off:off + 1024].rearrange(
                    "p (i2 t) -> p i2 t", i2=2))
        nc.vector.memset(vtpk[:, :, :, :, D:DA], vsc)

        # int32 scalar constants for the DVE rsqrt bit-trick
        I32 = mybir.dt.int32
        shr1 = singles.tile([P, 1], I32)
        nc.vector.memset(shr1, 1)
        mall = singles.tile([P, 1], I32)
        nc.vector.memset(mall, -1)
        magic = singles.tile([P, 16], I32)
        nc.vector.memset(magic, 0x5f3759e0)

        # ---------------- helpers ----------------
        def ln_stats(src_ap, mv_slice):
            st = work.tile([P, 6], F32, tag="bnst")
            nc.vector.bn_stats(out=st, in_=src_ap)
            nc.vector.bn_aggr(out=mv_slice, in_=st)

        def ln_rstd(mv_ap, rstd_ap, n, tag):
            """rstd = rsqrt(var + eps) on DVE: bit-trick seed + 2 Newton."""
            va = work.tile([P, n], F32, tag=f"{tag}va")
            nc.vector.tensor_scalar(out=va, in0=mv_ap, scalar1=EPS, scalar2=None,
                                    op0=OP.add)
            vi = va.bitcast(mybir.dt.int32)
            ri = rstd_ap.bitcast(mybir.dt.int32)
            nc.vector.tensor_scalar(out=ri, in0=vi, scalar1=shr1[:, 0:1],
                                    scalar2=None, op0=OP.logical_shift_right)
            nc.vector.tensor_scalar(out=ri, in0=ri, scalar1=mall[:, 0:1],
                                    scalar2=None, op0=OP.bitwise_xor)
            nc.vector.tensor_tensor(out=ri, in0=ri, in1=magic[:, 0:n],
                                    op=OP.add)
            t = work.tile([P, n], F32, tag=f"{tag}t")
            for _ in range(2):
                nc.vector.tensor_tensor(out=t, in0=va, in1=rstd_ap, op=OP.mult)
                nc.vector.tensor_tensor(out=t, in0=t, in1=rstd_ap, op=OP.mult)
                nc.vector.tensor_scalar(out=t, in0=t, scalar1=-0.5,
                                        scalar2=1.5, op0=OP.mult, op1=OP.add)
                nc.vector.tensor_tensor(out=rstd_ap, in0=rstd_ap, in1=t,
                                        op=OP.mult)

        def z_and_pack(x_ap, mv_slice, rstd_ap, dst, col, ztag):
            """LN z (DVE, bf16) -> 2 strided transposes -> fp8 pair cast."""
            z = work.tile([P, D], BF16, tag=ztag)
            nc.vector.tensor_scalar(
                out=z, in0=x_ap, scalar1=mv_slice[:, 0:1], scalar2=rstd_ap,
                op0=OP.subtract, op1=OP.mult)
            tp = pst.tile([P, 2, P], BF16, tag="tp")
            nc.tensor.transpose(tp[0:96, 0, :], z[:, 0:D:2], ident)
            nc.tensor.transpose(tp[0:96, 1, :], z[:, 1:D:2], ident)
            nc.vector.tensor_copy(out=dst[0:96, :, col:col + P], in_=tp[0:96, :, :])

        def emit_sc(c, g, w):
            """scores for (chunk c, head g, s-block w) -> psum pair + exp.

            The token axis is globally even/odd permuted within 256-blocks
            (tpos = 256w + 128*par + k <-> t = 256w + 2k + par), so the
            pair-packed layouts come from CONTIGUOUS lhsT column slices.
            """
            pp = ps2.tile([P, 2, TCH], F32, tag="p2")
            base = 256 * w
            csl = slice(c * TCH, (c + 1) * TCH)
            for par in range(2):
                nc.tensor.matmul(
                    pp[:, par, :],
                    lhsT=zpk[0:KP, :, base + 128 * par:base + 128 * par + P],
                    rhs=nhpk[0:KP, :, g, csl],
                    start=True, stop=True, perf_mode=PM.DoubleRow)
            et = e_pool.tile([P, 2, TCH], FP8, tag=f"e{g}_{w}")
            nc.scalar.activation(out=et, in_=pp, func=AF.Exp, scale=1.0 / gsc)
            return et

        def emit_vt(h, w):
            for par in range(2):
                pv = psc.tile([P, TCH], F32, tag="pc")
                base = 256 * w
                nc.tensor.matmul(
                    pv[:, 0:D],
                    lhsT=zpk[0:KP, :, base + 128 * par:base + 128 * par + P],
                    rhs=vsb[0:KP, :, h, :],
                    start=True, stop=True, perf_mode=PM.DoubleRow)
                nc.vector.tensor_copy(out=vtpk[:, h, w, par, 0:D], in_=pv[:, 0:D])

        def emit_nh(g, q, copy_eng):
            for par in range(2):
                mw = KP if par == 0 else KP - 1
                pn = psc.tile([P, TCH], F32, tag="pc")
                nc.tensor.matmul(
                    pn[0:mw, :],
                    lhsT=gsb[0:KP, :, g, KP * par:KP * par + mw],
                    rhs=zpk[0:KP, :, q * TCH:(q + 1) * TCH],
                    start=True, stop=True, perf_mode=PM.DoubleRow)
                dst = nhpk[0:mw, par, g, q * TCH:(q + 1) * TCH]
                if copy_eng == "act":
                    nc.scalar.copy(out=dst, in_=pn[0:mw, :])
                else:
                    nc.vector.tensor_copy(out=dst, in_=pn[0:mw, :])

        def emit_ctx_group(c, e_tiles, h, ts):
            """ctx accumulation over 8 s-blocks + y1 update for (h, ts)."""
            ti = c * TSUB + ts
            pc = psc.tile([P, TCH], F32, tag="pc")
            for w in range(NW):
                nc.tensor.matmul(
                    pc[:, 0:DA],
                    lhsT=e_tiles[(h, w)][:, :, ts * P:(ts + 1) * P],
                    rhs=vtpk[0:P, h, w, :, 0:DA],
                    start=(w == 0), stop=(w == NW - 1),
                    perf_mode=PM.DoubleRow)
            rc = work.tile([P, 1], F32, tag=f"rc{ts}")
            nc.vector.reciprocal(out=rc, in_=pc[:, D:DA])
            nc.vector.scalar_tensor_tensor(
                out=y1_tiles[ti], in0=pc[:, 0:D], scalar=rc,
                in1=(xall[:, ti, :] if h == 0 else y1_tiles[ti]),
                op0=OP.mult, op1=OP.add)

        def emit_ln2(c, mv2, rstd2):
            for ts in range(TSUB):
                ln_stats(y1_tiles[c * TSUB + ts], mv2[:, ts, :])
            ln_rstd(mv2[:, :, 1], rstd2, TSUB, f"ln2_{c % 2}")
            for ts in range(TSUB):
                z_and_pack(y1_tiles[c * TSUB + ts], mv2[:, ts, :],
                           rstd2[:, ts:ts + 1], n2t[c % 2], ts * P, f"z2_{ts}")

        def emit_fc1(c):
            for B in range(NB):
                pp = ps2.tile([P, 2, TCH], F32, tag="p2")
                for par in range(2):
                    nc.tensor.matmul(
                        pp[:, par, :],
                        lhsT=w1sb[0:KP, :, 256 * B + 128 * par:
                                  256 * B + 128 * par + P],
                        rhs=n2t[c % 2][0:KP, :, :],
                        start=True, stop=True, perf_mode=PM.DoubleRow)
                yield B, pp

        def emit_gelu(c, B, pp):
            nc.scalar.activation(out=ht[c % 2][B], in_=pp, func=AF.Gelu,
                                 scale=1.0 / w1sc)

        def emit_fc2(c):
            for ts in range(TSUB):
                ti = c * TSUB + ts
                pf = psc.tile([P, TCH], F32, tag="pc")
                for B in range(NB):
                    nc.tensor.matmul(
                        pf[:, 0:D],
                        lhsT=ht[c % 2][B][:, :, ts * P:(ts + 1) * P],
                        rhs=w2sb[:, B, :, :],
                        start=(B == 0), stop=(B == NB - 1),
                        perf_mode=PM.DoubleRow)
                ot = work.tile([P, D], F32, tag=f"ot{ts}")
                nc.vector.scalar_tensor_tensor(
                    out=ot, in0=pf[:, 0:D], scalar=1.0 / w2sc,
                    in1=y1_tiles[ti], op0=OP.mult, op1=OP.add)
                if has_b2:
                    nc.vector.tensor_tensor(out=ot, in0=ot, in1=b2sb, op=OP.add)
                nc.sync.dma_start(out=y_perm[ti // 2, ti % 2], in_=ot)

        # ---------------- Phase A: x load + LN1 + zpk ----------------
        # Global token permutation: tpos = 256w + 128*par + k <-> t = 256w
        # + 2k + par.  x is loaded permuted; y is stored back permuted; all
        # intermediate stages are token-oblivious.
        x_perm = x_d.ap().rearrange("(w k two) d -> w k two d", two=2, k=P)
        y_perm = y_d.ap().rearrange("(w k two) d -> w two k d", two=2, k=P)
        for w in range(NW):
            nc.sync.dma_start(out=xall[:, 2 * w:2 * w + 2, :], in_=x_perm[w])
        for g4 in range(4):
            for i in range(4 * g4, 4 * g4 + 4):
                ln_stats(xall[:, i, :], mv1[:, i, :])
            ln_rstd(mv1[:, 4 * g4:4 * (g4 + 1), 1],
                    rstd1[:, 4 * g4:4 * (g4 + 1)], 4, f"ln1_{g4 % 2}")
            for i in range(4 * g4, 4 * g4 + 4):
                z_and_pack(xall[:, i, :], mv1[:, i, :], rstd1[:, i:i + 1],
                           zpk, i * P, f"z1_{i % 4}")

        # ---------------- Phase B: nh for g0 (rest woven into W0) -----
        for q in range(NCH):
            emit_nh(0, q, "act")

        # ---------------- chunk windows ----------------
        mv2 = [work.tile([P, TSUB, 2], F32, tag=f"mv2_{i}", name=f"mv2_{i}")
               for i in range(2)]
        rstd2 = [work.tile([P, TSUB], F32, tag=f"rs2_{i}", name=f"rs2_{i}")
                 for i in range(2)]
        e_tiles_by_chunk = {}

        # W0: scores/exp(0); weave nh(g1,g2) + vt jobs
        fill_w0 = ([("nh", 1, q) for q in range(NCH)]
                   + [("nh", 2, q) for q in range(NCH)]
                   + [("vt", h, w) for h in range(NHEAD) for w in range(NW)])
        et0 = {}
        fi = 0
        for g in range(NHEAD):
            for w in range(NW):
                et0[(g, w)] = emit_sc(0, g, w)
                take = 2 if fi < 8 else 1
                for _ in range(take):
                    if fi < len(fill_w0):
                        job = fill_w0[fi]; fi += 1
                        if job[0] == "nh":
                            emit_nh(job[1], job[2], "dve")
                        else:
                            emit_vt(job[1], job[2])
        while fi < len(fill_w0):
            job = fill_w0[fi]; fi += 1
            if job[0] == "nh":
                emit_nh(job[1], job[2], "dve")
            else:
                emit_vt(job[1], job[2])
        e_tiles_by_chunk[0] = et0

        # W1..W3: scores/exp(c) woven with ctx(c-1); then LN2(c-1); MLP spread
        for c in range(1, NCH):
            etc = {}
            prev = e_tiles_by_chunk[c - 1]
            ctx_groups = [(h, ts) for h in range(NHEAD) for ts in range(TSUB)]
            ci = 0
            sc_cnt = 0
            for g in range(NHEAD):
                for w in range(NW):
                    etc[(g, w)] = emit_sc(c, g, w)
                    sc_cnt += 1
                    if sc_cnt % 2 == 0 and ci < len(ctx_groups):
                        h, ts = ctx_groups[ci]; ci += 1
                        emit_ctx_group(c - 1, prev, h, ts)
            while ci < len(ctx_groups):
                h, ts = ctx_groups[ci]; ci += 1
                emit_ctx_group(c - 1, prev, h, ts)
            e_tiles_by_chunk[c] = etc
            del e_tiles_by_chunk[c - 1]

            emit_ln2(c - 1, mv2[(c - 1) % 2], rstd2[(c - 1) % 2])

            if c == 2:
                # MLP(0): fc1+gelu now; fc2(0) next window
                for B, pp in emit_fc1(0):
                    emit_gelu(0, B, pp)
                for B, pp in emit_fc1(1):
                    emit_gelu(1, B, pp)
            if c == 3:
                emit_fc2(0)
                emit_fc2(1)

        # tail: ctx(3), LN2(3), MLP(2), MLP(3)
        prev = e_tiles_by_chunk[3]
        for h in range(NHEAD):
            for ts in range(TSUB):
                emit_ctx_group(3, prev, h, ts)
        emit_ln2(3, mv2[1], rstd2[1])
        for B, pp in emit_fc1(2):
            emit_gelu(2, B, pp)
        emit_fc2(2)
        for B, pp in emit_fc1(3):
            emit_gelu(3, B, pp)
        emit_fc2(3)

    nc.finalize()
    return nc


_module_cache = {}


def kernel(**inputs):
    global LAST_RESULTS
    x = np.ascontiguousarray(np.asarray(inputs["x"], np.float32))
    B = x.shape[0]
    assert x.shape == (B, T, D) and B == 8

    weights, scales, has_b2 = _prep_host(inputs)

    _install_table_patch()
    key = (scales, has_b2)
    if key not in _module_cache:
        _module_cache[key] = _build(scales, has_b2)
    nc = _module_cache[key]

    in_maps = [dict(weights, x=x[b]) for b in range(B)]
    res = run_bass_kernel_spmd(nc, in_maps, core_ids=list(range(B)), trace=TRACE)
    LAST_RESULTS = res
    out = np.stack([np.asarray(res.results[b]["y"], np.float32) for b in range(B)])
    return out
